# revision 33
# baseline (speedup 1.0000x reference)
"""Multi-head self-attention with RoPE, sharded over 8 TRN2 NeuronCores.

Sharding: tensor-parallel over heads (2 heads/core) for QKV projections and
attention; an AllToAll redistributes attention outputs from head-sharded to
sequence-sharded so each core computes 1/8 of the output projection rows.

Device-side layout choices (host pre-stages everything):
- x is passed transposed (xt = x.T) so projection matmuls contract naturally.
- Wq/Wk rows are pair-permuted (evens then odds per head) so RoPE becomes
  rotate-half form; the 1/sqrt(hd) score scale is folded into Wq.
- Scores are computed transposed (S^T = K @ Q^T, keys on partitions) so the
  softmax denominator comes free from an ones-column appended to V, and P^T
  feeds the PV matmul with no on-device transpose of P.
- All matmuls run as float32r (full PE rate, ~fp22 multiply precision).

Hardcoded problem shape: B=1, S=4096, D=1024, H=16, hd=64, theta=10000.
"""

import math

import numpy as np

import concourse.bass as bass
import concourse.mybir as mybir
import concourse.tile as tile
from concourse import bacc
from concourse.bass_utils import run_bass_kernel_spmd

N_CORES = 8
D_MODEL = 1024
NUM_HEADS = 16
HEAD_DIM = 64
THETA = 10000.0
P = 128  # partitions; also = 2 heads x 64 dims per core
KD = D_MODEL // 128  # 8 contraction tiles for the projections

F32 = mybir.dt.float32
F32R = mybir.dt.float32r
BF16 = mybir.dt.bfloat16
EXP = mybir.ActivationFunctionType.Exp

ATTN_BF16 = True  # bf16 for the attention matmul path (x, Wqkv, Q/K, V, P)
ADT = BF16 if ATTN_BF16 else F32R


def build(seq: int, p12_reps: int = 1, p3_reps: int = 1, parts: str = "full"):
    """Build the SPMD Bass program for sequence length `seq`.

    p12_reps > 1 wraps phases 1+2 (projections + attention) in an on-device
    For_i loop; p3_reps > 1 unrolls phase 3 (A2A + out-proj) — both exist
    for wall-clock timing above the axon dispatch floor. Defaults give the
    normal single-shot kernel.
    """
    CH = min(512, seq)          # free-dim chunk for matmuls / PSUM banks
    NCH = seq // CH             # number of seq chunks
    KB = seq // 128             # key blocks
    KBC = CH // 128             # key blocks per chunk (4 at CH=512)
    SW = seq // N_CORES         # per-core output seq shard
    SPC = CH // SW if CH >= SW else 1  # a2a shards per chunk

    nc = bacc.Bacc("TRN2", num_devices=N_CORES)

    xt = nc.dram_tensor("xt", [D_MODEL, seq], ADT, kind="ExternalInput")
    wq = nc.dram_tensor("wq", [P, D_MODEL], ADT, kind="ExternalInput")
    wk = nc.dram_tensor("wk", [P, D_MODEL], ADT, kind="ExternalInput")
    wv = nc.dram_tensor("wv", [P, D_MODEL], ADT, kind="ExternalInput")
    wqs = nc.dram_tensor("wqs", [P, D_MODEL], ADT, kind="ExternalInput")
    wks = nc.dram_tensor("wks", [P, D_MODEL], ADT, kind="ExternalInput")
    wo = nc.dram_tensor("wo", [P, KD * D_MODEL], F32R, kind="ExternalInput")
    ctab = nc.dram_tensor("ctab", [P, seq], F32, kind="ExternalInput")
    stab = nc.dram_tensor("stab", [P, seq], F32, kind="ExternalInput")
    dmaskd = nc.dram_tensor("dmask", [P, (CH // 128) * CH], F32,
                            kind="ExternalInput")
    ident = nc.dram_tensor("ident", [P, 128], F32, kind="ExternalInput")
    onesd = nc.dram_tensor("ones", [P, max(KB, 64)], F32, kind="ExternalInput")
    zerod = nc.dram_tensor("zeros", [P + 2, SW], F32R, kind="ExternalInput")
    out_d = nc.dram_tensor("out", [D_MODEL, SW], F32, kind="ExternalOutput")

    with tile.TileContext(nc) as tc:
        with (
            tc.tile_pool(name="const", bufs=1) as cpool,
            tc.tile_pool(name="mats", bufs=1) as mpool,
            tc.tile_pool(name="xt", bufs=2) as xpool,
            tc.tile_pool(name="sc", bufs=2) as spool,
            tc.tile_pool(name="pt", bufs=6) as ptpool,
            tc.tile_pool(name="wo", bufs=3) as wopool,
            tc.tile_pool(name="ps", bufs=2, space="PSUM") as pspool,
            tc.tile_pool(name="pss", bufs=2, space="PSUM") as psspool,
            tc.tile_pool(name="dram", bufs=1, space="DRAM") as dpool,
        ):
            # ---- constants ----
            w_sb = {}
            wlist = [("q", wq), ("k", wk), ("v", wv)]
            if "peswap" in parts:
                wlist += [("qs", wqs), ("ks", wks)]
            for name, src in wlist:
                t = cpool.tile([P, D_MODEL], ADT, tag=f"w{name}")
                nc.sync.dma_start(out=t[:], in_=src[:])
                w_sb[name] = t
            ct = cpool.tile([P, seq], F32, tag="ct")
            nc.sync.dma_start(out=ct[:], in_=ctab[:])
            st = cpool.tile([P, seq], F32, tag="st")
            nc.sync.dma_start(out=st[:], in_=stab[:])
            dmask = cpool.tile([P, KBC * CH], F32, tag="dmask")
            nc.sync.dma_start(out=dmask[:], in_=dmaskd[:])
            idn = cpool.tile([P, 128], F32, tag="idn")
            nc.sync.dma_start(out=idn[:], in_=ident[:])
            ones = cpool.tile([P, max(KB, 64)], F32, tag="ones")
            nc.sync.dma_start(out=ones[:], in_=onesd[:])
            e01d = nc.dram_tensor("e01", [2, P], F32R, kind="ExternalInput")
            e01 = cpool.tile([2, P], F32R, tag="e01")
            nc.sync.dma_start(out=e01[:], in_=e01d[:])

            # ---- persistent matrices ----
            qT = mpool.tile([P, seq], ADT, tag="qT")  # rows: 2 heads x 64
            kT = mpool.tile([P, seq], ADT, tag="kT")
            vnat = mpool.tile([P, KB * 130], ADT, tag="vnat")

            a2a_in1 = dpool.tile([N_CORES, P + 2, SW], F32R, tag="a2a_in1")
            a2a_out1 = dpool.tile([N_CORES, P + 2, SW], F32R,
                                  tag="a2a_out1")
            a2a_in2 = dpool.tile([N_CORES, P + 2, SW], F32R, tag="a2a_in2")
            a2a_out2 = dpool.tile([N_CORES, P + 2, SW], F32R,
                                  tag="a2a_out2")
            NSH = N_CORES * SPC // (1 if SPC > 1 else 1)  # total shards = 8
            HALF = N_CORES // 2
            # zero the never-written halves once (DRAM->DRAM copies)
            for s_ in range(N_CORES):
                dst = a2a_in1 if s_ >= HALF else a2a_in2
                nc.sync.dma_start(out=dst[s_], in_=zerod[:])

            def emit_proj_chunk(sc):
                """Projections + rope + V-transpose for seq chunk sc."""
                sl = bass.ts(sc, CH)
                xts = []
                for k in range(KD):
                    t = xpool.tile([P, CH], ADT, tag=f"xt{k}",
                                   name=f"xt_{sc}_{k}")
                    nc.sync.dma_start(
                        out=t[:], in_=xt[128 * k:128 * (k + 1), sl]
                    )
                    xts.append(t)
                if parts == "dma":
                    return
                vt_c = spool.tile([P, CH], F32, tag="vt")
                sw_c = {}
                for nm in ("qs", "ks"):
                    sw_c[nm] = spool.tile([P, CH], F32, tag=nm,
                                          name=f"sw_{sc}_{nm}")
                projs = [("q", qT[:, sl]), ("k", kT[:, sl]), ("v", vt_c[:])]
                if "peswap" in parts:
                    projs += [("qs", sw_c["qs"][:]), ("ks", sw_c["ks"][:])]
                for name, dst in projs:
                    ps = pspool.tile([P, CH], F32, tag="mm",
                                     name=f"proj_{sc}_{name}")
                    for k in range(KD):
                        nc.tensor.matmul(
                            ps[:],
                            w_sb[name][:, bass.ts(k, 128)],
                            xts[k][:],
                            start=(k == 0),
                            stop=(k == KD - 1),
                        )
                    nc.vector.tensor_copy(dst, ps[:])

                if parts == "proj":
                    return
                # rope: mat = mat*cos + swapped*sin; the swapped-halves
                # version comes from PE (wqs/wks projection) or DVE copies
                for mi, (mat, swc) in ((0, (qT, sw_c["qs"])),
                                       (1, (kT, sw_c["ks"]))):
                    if "peswap" not in parts:
                        for h in (0, 1):
                            for half in (0, 1):
                                d0 = 64 * h + 32 * half
                                s0 = 64 * h + 32 * (1 - half)
                                nc.vector.tensor_copy(
                                    swc[d0:d0 + 32, :], mat[s0:s0 + 32, sl]
                                )
                    tm = spool.tile([P, CH], F32, tag="tmp",
                                    name=f"tmp_{sc}_{mi}")
                    nc.vector.tensor_mul(tm[:], swc[:], st[:, sl])
                    nc.vector.tensor_mul(mat[:, sl], mat[:, sl], ct[:, sl])
                    nc.vector.tensor_add(mat[:, sl], mat[:, sl], tm[:])

                if parts == "rope":
                    return
                # V transpose for this chunk's key blocks
                for j in range(KBC):
                    kb = sc * KBC + j
                    pst = pspool.tile([P, CH], F32, tag="mm",
                                      name=f"vtr_{kb}")
                    nc.tensor.transpose(
                        pst[:, 0:128], vt_c[:, bass.ts(j, 128)], idn[:]
                    )
                    nc.vector.tensor_copy(
                        vnat[:, 130 * kb:130 * kb + 64], pst[:, 0:64]
                    )
                    nc.vector.tensor_copy(
                        vnat[:, 130 * kb + 65:130 * kb + 129],
                        pst[:, 64:128]
                    )

            def emit_attn_chunk(qc):
                """Attention for query chunk qc (needs proj chunks 0..qc).

                Per key block: S^T for both heads lands in one [128, 2*CH]
                PSUM tile ([0:CH]=h0, [CH:2CH]=h1) so a single wide exp
                covers both heads. Diagonal-band blocks are processed FIRST
                (their masking runs on GPSIMD and gets latency-hidden behind
                the non-diagonal tail of the PV accumulation).
                """
                kbmax = (qc + 1) * KBC
                psu = {}
                for h in (0, 1):
                    psu[h] = pspool.tile([65, CH], F32, tag="u",
                                         name=f"psu_{qc}_{h}")
                kb_order = (list(range(kbmax - KBC, kbmax))
                            + list(range(0, kbmax - KBC)))

                def emit_pv(kb, ki, pt):
                    for h in (0, 1):
                        nc.tensor.matmul(
                            psu[h][:],
                            vnat[:, 130 * kb + 65 * h:
                                 130 * kb + 65 * (h + 1)],
                            pt[:, CH * h:CH * (h + 1)],
                            start=(ki == 0),
                            stop=(ki == kbmax - 1),
                        )

                pending = None  # (kb, ki, pt) whose PV is deferred one step
                for ki, kb in enumerate(kb_order):
                    j = kb - (kbmax - KBC)  # diag index if >= 0
                    pss = psspool.tile([P, 2 * CH], F32, tag="s",
                                       name=f"sc_{qc}_{kb}")
                    for h in (0, 1):
                        nc.tensor.matmul(
                            pss[:, CH * h:CH * (h + 1)],
                            kT[64 * h:64 * (h + 1), bass.ts(kb, 128)],
                            qT[64 * h:64 * (h + 1), bass.ts(qc, CH)],
                            start=True,
                            stop=True,
                        )
                    pt = ptpool.tile([P, 2 * CH], ADT, tag="pt",
                                     name=f"pt_{qc}_{kb}")
                    nc.scalar.activation(pt[:], pss[:], EXP)
                    if j >= 0:
                        # zero q < key region: cols [0, 128j) fully +
                        # triangle at [128j, 128j+128), per head half
                        w = 128 * (j + 1)
                        meng = nc.gpsimd if "gmask" in parts else nc.vector
                        for h in (0, 1):
                            meng.tensor_mul(
                                pt[:, CH * h:CH * h + w],
                                pt[:, CH * h:CH * h + w],
                                dmask[:, CH * j:CH * j + w],
                            )
                    if parts == "attn_s":
                        continue
                    if "nodefer" in parts:
                        emit_pv(kb, ki, pt)
                        continue
                    if pending is not None:
                        emit_pv(*pending)
                    pending = (kb, ki, pt)
                if parts not in ("attn_s", "attn_pv", "nodefer") and pending is not None:
                    emit_pv(*pending)
                if parts in ("attn_s", "attn_pv"):
                    return
                # epilogue: extract U (unnormalized) and sums; the
                # normalization happens post-A2A on the out-proj side
                for h in (0, 1):
                    ut = spool.tile([65, CH], F32R, tag=f"ut{h}",
                                    name=f"ut_{qc}_{h}")
                    nc.vector.tensor_copy(ut[:], psu[h][:])
                    for jj in range(SPC):
                        shard = qc * SPC + jj
                        tgt = a2a_in1 if shard < HALF else a2a_in2
                        nc.sync.dma_start(
                            out=tgt[shard, 64 * h:64 * (h + 1), :],
                            in_=ut[0:64, SW * jj:SW * (jj + 1)],
                        )
                        nc.sync.dma_start(
                            out=tgt[shard, P + h:P + h + 1, :],
                            in_=ut[64:65, SW * jj:SW * (jj + 1)],
                        )

            def emit_p12():
                # ones columns (cols 64 and 129 of each 130-wide block) via
                # strided DVE copies so the producer output dtype is f32r
                vv = vnat[:].rearrange("p (k c) -> p k c", c=130)
                oo = ones[:, 0:KB].rearrange("p (k c) -> p k c", c=1)
                nc.vector.tensor_copy(vv[:, :, 64:65], oo)
                nc.vector.tensor_copy(vv[:, :, 129:130], oo)
                last_half_qc = (HALF - 1) // SPC  # qc completing shard HALF-1
                for sc in range(NCH):
                    emit_proj_chunk(sc)
                    if parts.split("_")[0] not in ("dma", "proj", "rope", "vt", "noil"):
                        emit_attn_chunk(sc)
                        if sc == last_half_qc and p12_reps == 1:
                            emit_cc(a2a_in1, a2a_out1)
                if parts == "noil":
                    for sc in range(NCH):
                        emit_attn_chunk(sc)

            def emit_cc(ab_in, ab_out):
                nc.gpsimd.collective_compute(
                    "AllToAll",
                    mybir.AluOpType.bypass,
                    replica_groups=[list(range(N_CORES))],
                    ins=[ab_in.opt()],
                    outs=[ab_out.opt()],
                )

            def emit_p3(cc1_done):
                if not cc1_done:
                    emit_cc(a2a_in1, a2a_out1)
                emit_cc(a2a_in2, a2a_out2)
                ats = []
                for i in range(N_CORES):
                    at = xpool.tile([P, SW], F32R, tag=f"xt{i}",
                                    name=f"at_{i}")
                    nc.sync.dma_start(out=at[:], in_=a2a_out1[i, 0:P, :])
                    at2 = spool.tile([P, SW], F32R, tag="at2",
                                     name=f"at2_{i}")
                    nc.sync.dma_start(out=at2[:], in_=a2a_out2[i, 0:P, :])
                    nc.vector.tensor_add(at[:], at[:], at2[:])
                    sm2 = spool.tile([2, SW], F32R, tag="sm2",
                                     name=f"sm2_{i}")
                    nc.sync.dma_start(out=sm2[:], in_=a2a_out1[i, P:P + 2, :])
                    sm2b = spool.tile([2, SW], F32R, tag="sm2b",
                                      name=f"sm2b_{i}")
                    nc.sync.dma_start(out=sm2b[:],
                                      in_=a2a_out2[i, P:P + 2, :])
                    nc.vector.tensor_add(sm2[:], sm2[:], sm2b[:])
                    rs2 = spool.tile([2, SW], F32R, tag="rs2",
                                     name=f"rs2_{i}")
                    with nc.allow_low_precision(
                            reason="f32r tag only; PE rounds on read"):
                        nc.vector.reciprocal(rs2[:], sm2[:])
                    rb = pspool.tile([P, SW], F32, tag="mm",
                                     name=f"rb_{i}")
                    nc.tensor.matmul(rb[:], e01[:], rs2[:],
                                     start=True, stop=True)
                    nc.vector.tensor_mul(at[:], at[:], rb[:])
                    ats.append(at)
                for e in range(KD):
                    wot = wopool.tile([P, D_MODEL], F32R, tag="wo",
                                      name=f"wot_{e}")
                    nc.sync.dma_start(
                        out=wot[:], in_=wo[:, bass.ts(e, D_MODEL)]
                    )
                    pso = pspool.tile([P, SW], F32, tag="mm",
                                      name=f"pso_{e}")
                    for i in range(N_CORES):
                        nc.tensor.matmul(
                            pso[:],
                            wot[:, bass.ts(i, 128)],
                            ats[i][:],
                            start=(i == 0),
                            stop=(i == N_CORES - 1),
                        )
                    ot = ptpool.tile([P, SW], F32, tag="pt",
                                     name=f"ot_{e}")
                    nc.vector.tensor_copy(ot[:], pso[:])
                    nc.sync.dma_start(out=out_d[bass.ts(e, 128)], in_=ot[:])

            if p12_reps == 1:
                emit_p12()
            else:
                with tc.For_i(0, p12_reps, 1):
                    emit_p12()
            for r3 in range(p3_reps):
                emit_p3(cc1_done=(p12_reps == 1 and r3 == 0))

    nc.finalize()
    return nc


def prepare_in_maps(in_features, token_positions, Wq, Wk, Wv, Wo, seq):
    """Host-side staging: shard/transform full inputs into per-core maps."""
    import ml_dtypes
    adt = ml_dtypes.bfloat16 if ATTN_BF16 else np.float32
    x = np.ascontiguousarray(np.asarray(in_features, dtype=np.float32)[0])
    pos = np.asarray(token_positions).reshape(-1)[:seq].astype(np.float64)

    xt = np.ascontiguousarray(x.T)  # [D, S]

    # RoPE tables in rotate-half form after pair permutation.
    inv_freq = THETA ** (-np.arange(0, HEAD_DIM, 2, dtype=np.float64)
                         / HEAD_DIM)
    ang = pos[:, None] * inv_freq[None, :]  # [S, 32]
    cos = np.cos(ang).T.astype(np.float32)  # [32, S]
    sin = np.sin(ang).T.astype(np.float32)
    ctab = np.ascontiguousarray(np.tile(cos, (4, 1)))  # [128, S]
    stab = np.ascontiguousarray(
        np.concatenate([-sin, sin, -sin, sin], axis=0)
    ).astype(np.float32)

    perm = np.concatenate(
        [np.arange(0, HEAD_DIM, 2), np.arange(1, HEAD_DIM, 2)]
    )  # within-head: evens then odds
    swap = np.concatenate([np.arange(32, 64), np.arange(0, 32)])
    perm_s = perm[swap]  # swapped-halves permutation composed with perm

    CH = min(512, seq)
    KBC = CH // 128
    tri = np.triu(np.ones((128, 128), dtype=np.float32))
    dmask = np.ones((128, KBC * CH), dtype=np.float32)
    for j in range(KBC):
        dmask[:, CH * j:CH * j + 128 * j] = 0.0
        dmask[:, CH * j + 128 * j:CH * j + 128 * (j + 1)] = tri
    ident = np.eye(128, dtype=np.float32)
    ones = np.ones((128, max(seq // 128, 64)), dtype=np.float32)
    e01_host = np.zeros((2, 128), dtype=np.float32)
    e01_host[0, 0:64] = 1.0
    e01_host[1, 64:128] = 1.0

    WoT = np.ascontiguousarray(np.asarray(Wo, dtype=np.float32).T)  # [d, e]
    wo_packed = np.empty((128, KD * D_MODEL), dtype=np.float32)
    for e in range(KD):
        for i in range(KD):
            wo_packed[:, D_MODEL * e + 128 * i: D_MODEL * e + 128 * (i + 1)] \
                = WoT[128 * i:128 * (i + 1), 128 * e:128 * (e + 1)]

    def pack_w(Wc):
        # Wc: [128 out, 1024 in] -> WT [1024, 128] -> [128, 8*128] k-tiled
        WT = np.ascontiguousarray(Wc.T)
        return np.ascontiguousarray(
            WT.reshape(KD, 128, 128).transpose(1, 0, 2).reshape(128, KD * 128)
        ).astype(np.float32)

    in_maps = []
    for c in range(N_CORES):
        rows = slice(128 * c, 128 * (c + 1))
        Wq_r = np.asarray(Wq, dtype=np.float32)[rows].reshape(2, 64, D_MODEL)
        Wq_c = (Wq_r[:, perm, :] / math.sqrt(HEAD_DIM)).reshape(128, D_MODEL)
        Wqs_c = (Wq_r[:, perm_s, :] / math.sqrt(HEAD_DIM)).reshape(128,
                                                                   D_MODEL)
        Wk_r = np.asarray(Wk, dtype=np.float32)[rows].reshape(2, 64, D_MODEL)
        Wk_c = Wk_r[:, perm, :].reshape(128, D_MODEL)
        Wks_c = Wk_r[:, perm_s, :].reshape(128, D_MODEL)
        Wv_c = np.asarray(Wv, dtype=np.float32)[rows]
        in_maps.append({
            "xt": xt.astype(adt),
            "wq": pack_w(Wq_c).astype(adt),
            "wk": pack_w(Wk_c).astype(adt),
            "wv": pack_w(Wv_c).astype(adt),
            "wqs": pack_w(Wqs_c).astype(adt),
            "wks": pack_w(Wks_c).astype(adt),
            "wo": wo_packed,
            "ctab": ctab,
            "stab": stab,
            "dmask": dmask,
            "ident": ident,
            "ones": ones,
            "zeros": np.zeros((130, seq // 8), dtype=np.float32),
            "e01": e01_host,
        })
    return in_maps


_BUILD_CACHE = {}


def _get_nc(seq, p12_reps=1, p3_reps=1, parts="full"):
    key = (seq, p12_reps, p3_reps, parts)
    if key not in _BUILD_CACHE:
        _BUILD_CACHE[key] = build(seq, p12_reps, p3_reps, parts)
    return _BUILD_CACHE[key]


def postprocess(results, seq, in_dtype):
    SW = seq // N_CORES
    out = np.empty((seq, D_MODEL), dtype=np.float32)
    for c in range(N_CORES):
        out[SW * c:SW * (c + 1), :] = results[c]["out"].T
    return out.reshape(1, seq, D_MODEL).astype(in_dtype)


def kernel(in_features, token_positions, Wq, Wk, Wv, Wo):
    in_dtype = np.asarray(in_features).dtype
    B, S, D = np.asarray(in_features).shape
    assert B == 1 and D == D_MODEL

    nc = _get_nc(S)
    in_maps = prepare_in_maps(in_features, token_positions, Wq, Wk, Wv, Wo, S)
    res = run_bass_kernel_spmd(nc, in_maps, list(range(N_CORES)), trace=False)
    return postprocess(res.results, S, in_dtype)
